# revision 1
# baseline (speedup 1.0000x reference)
"""Trainium2 Bass kernel for nn_Attention_15865609191618 (sparse_attention).

Reference computation per sequence s (4096 sequences of shape [n=64, dim=128]):
    qkv = x @ W_qkv ; q,k,v split; 4 heads x 32
    sim = (q * 32**-0.5) @ k^T + pos_bias[h]
    attn = softmax(sim, -1)
    out = (attn @ v) @ W_out

Sharding: pure data parallel. b*b2 = 4096 sequences split across 8 cores
(512 each); weights + pos_bias replicated.

Per-core layout strategy (groups of 8 sequences; 64 groups):
  - sequences packed in PAIRS on the 128 partitions (s' in {0,1} -> rows
    64s'..64s'+63), 4 pairs per group.
  - all matmul inputs bf16 (PE runs 1 cycle/row vs 4 for fp32); PSUM f32.
  - transposes on the PE (128x128 pair blocks).
  - pos_bias added by accumulating an extra matmul (I2 stacked identity
    lhsT x pos_bias rhs) onto the sim PSUM bank.
  - softmax without max-subtraction (values are small: |sim| < ~30 before
    exp, safely inside fp32 exp range); exp on ScalarE directly
    PSUM->SBUF; row-sums on GPSIMD; the 1/rowsum multiply is fused into
    the attn-out PSUM->SBUF copy on VectorE.
"""

import numpy as np

N_CORES = 8
B, B2, N, DIM = 4, 1024, 64, 128
HEADS, DH = 4, 32
HID = HEADS * DH
SEQS = B * B2            # 4096
SEQS_PER_CORE = SEQS // N_CORES  # 512
G = 8                    # sequences per group
NPAIR = G // 2           # 4 pairs
NGROUPS = SEQS_PER_CORE // G     # 64
SCALE = DH ** -0.5


def build_nc(ngroups=NGROUPS, stage=99):
    import concourse.bass as bass
    import concourse.mybir as mybir
    from concourse.tile import TileContext

    f32 = mybir.dt.float32
    bf16 = mybir.dt.bfloat16

    nc = bass.Bass()

    x_ext = nc.declare_dram_parameter("x", [SEQS_PER_CORE, N, DIM], f32, isOutput=False)
    pb_ext = nc.declare_dram_parameter("pos_bias", [HEADS, N, N], f32, isOutput=False)
    wqkv_ext = nc.declare_dram_parameter("W_qkv", [DIM, 3 * HID], f32, isOutput=False)
    wout_ext = nc.declare_dram_parameter("W_out", [HID, DIM], f32, isOutput=False)
    out_ext = nc.declare_dram_parameter("out", [SEQS_PER_CORE, N, DIM], f32, isOutput=True)

    with TileContext(nc) as tc:
        with (
            tc.tile_pool(name="singles", bufs=1) as singles,
            tc.tile_pool(name="work", bufs=2) as work,
            tc.tile_pool(name="ps1", bufs=6, space="PSUM") as ps1,
            tc.tile_pool(name="ps2", bufs=1, space="PSUM") as ps2,
        ):
            # ---------------- one-time constants ----------------
            w_f32 = singles.tile([DIM, 3 * HID], f32)
            nc.sync.dma_start(out=w_f32, in_=wqkv_ext[:, :])
            wo_f32 = singles.tile([HID, DIM], f32)
            nc.sync.dma_start(out=wo_f32, in_=wout_ext[:, :])
            pb_f32 = singles.tile([N, HEADS, N], f32)
            nc.sync.dma_start(out=pb_f32, in_=pb_ext.rearrange("h i j -> i h j"))

            wq_bf = singles.tile([DIM, HID], bf16)
            # fold the attention scale into W_q
            nc.vector.tensor_scalar_mul(wq_bf, w_f32[:, 0:HID], float(SCALE))
            wk_bf = singles.tile([DIM, HID], bf16)
            nc.vector.tensor_copy(wk_bf, w_f32[:, HID:2 * HID])
            wv_bf = singles.tile([DIM, HID], bf16)
            nc.vector.tensor_copy(wv_bf, w_f32[:, 2 * HID:3 * HID])
            wo_bf = singles.tile([HID, DIM], bf16)
            nc.vector.tensor_copy(wo_bf, wo_f32)
            pb_bf = singles.tile([N, HEADS, N], bf16)
            nc.vector.tensor_copy(pb_bf, pb_f32)

            ones_bf = singles.tile([128, 128], bf16)
            nc.vector.memset(ones_bf, 1.0)
            ident = singles.tile([128, 128], bf16)
            # value = -p + j ; keep where == 0 -> identity matrix
            nc.gpsimd.affine_select(
                ident, ones_bf, pattern=[[1, 128]],
                compare_op=mybir.AluOpType.is_equal, fill=0.0,
                base=0, channel_multiplier=-1,
            )
            # I2[k, (s', i)] = delta(k, i) for both halves  [64, 2*64]
            i2_bf = singles.tile([N, 2, N], bf16)
            nc.gpsimd.affine_select(
                i2_bf, ones_bf[0:N, :].rearrange("p (s i) -> p s i", s=2),
                pattern=[[0, 2], [1, N]],
                compare_op=mybir.AluOpType.is_equal, fill=0.0,
                base=0, channel_multiplier=-1,
            )

            # ---------------- main loop ----------------
            for g in range(ngroups):
                xg = x_ext[g * G:(g + 1) * G]  # [8, 64, 128]
                # pairs: seq = 2p + s'; partition = 64 s' + n; free = (p, d)
                xg_r = xg.rearrange("(p s) n d -> s n p d", s=2)

                x2 = work.tile([128, NPAIR, DIM], f32)
                nc.sync.dma_start(
                    out=x2.rearrange("(s n) p d -> s n p d", s=2), in_=xg_r)

                if stage < 1: continue
                x2b = work.tile([128, NPAIR, DIM], bf16)
                nc.vector.tensor_copy(x2b, x2)

                if stage < 2: continue
                # transpose pair blocks: [(s'n), d] -> [d, (s'n)]
                xT_ps = ps1.tile([128, NPAIR, 128], bf16, tag="psA")
                for p in range(NPAIR):
                    nc.tensor.transpose(xT_ps[:, p, :], x2b[:, p, :], ident)
                xT = work.tile([128, NPAIR, 128], bf16)
                nc.vector.tensor_copy(xT, xT_ps)
                xTf = xT.rearrange("d p m -> d (p m)")

                if stage < 3: continue
                # q/k projections, transposed layout: [hid, (p, s', n)]
                q_ps = ps1.tile([HID, NPAIR, 2, N], f32, tag="psA")
                nc.tensor.matmul(q_ps.rearrange("h p s n -> h (p s n)"),
                                 lhsT=wq_bf, rhs=xTf, start=True, stop=True)
                k_ps = ps1.tile([HID, NPAIR, 2, N], f32, tag="psA")
                nc.tensor.matmul(k_ps.rearrange("h p s n -> h (p s n)"),
                                 lhsT=wk_bf, rhs=xTf, start=True, stop=True)
                qT = work.tile([HID, NPAIR, 2, N], bf16)
                nc.scalar.copy(qT, q_ps)
                kT = work.tile([HID, NPAIR, 2, N], bf16)
                nc.vector.tensor_copy(kT, k_ps)

                if stage < 4: continue
                # v natural per pair: [(s', j), hid]
                v2_ps = ps1.tile([128, NPAIR, HID], f32, tag="psA")
                for p in range(NPAIR):
                    nc.tensor.matmul(v2_ps[:, p, :], lhsT=xT[:, p, :],
                                     rhs=wv_bf, start=True, stop=True)
                v2 = work.tile([128, NPAIR, HID], bf16)
                nc.vector.tensor_copy(v2, v2_ps)

                if stage < 5: continue
                # sim: per (pair, s', head) -> [64(i), 64(j)] block
                sim_ps = ps2.tile([128, NPAIR, HEADS, N], f32, tag="psS")
                import os as _os
                _hs = list(range(HEADS)) if _os.environ.get("SIM_ROWS") else ([0] if _os.environ.get("SIM_MIN") else list(range(HEADS)))
                _ss = [0] if (_os.environ.get("SIM_MIN") or _os.environ.get("SIM_ROWS")) else [0, 1]
                for p in range(NPAIR):
                    for s in _ss:
                        for h in _hs:
                            nc.tensor.matmul(
                                sim_ps[64 * s:64 * s + 64, p, h, :],
                                lhsT=qT[32 * h:32 * h + 32, p, s, :],
                                rhs=kT[32 * h:32 * h + 32, p, s, :],
                                start=True, stop=True,
                                tile_position=(32 * h, 64 * s),
                                skip_group_check=True,
                            )
                    # accumulate pos_bias onto both seq halves of this pair
                    import os
                    if os.environ.get("SKIP_PB"): continue
                    nc.tensor.matmul(
                        sim_ps[:, p, :, :].rearrange("m h j -> m (h j)"),
                        lhsT=i2_bf.rearrange("k s i -> k (s i)"),
                        rhs=pb_bf.rearrange("k h j -> k (h j)"),
                        start=False, stop=True,
                        skip_group_check=True,
                    )

                if stage < 6: continue
                # exp (no max subtraction; fp32-safe range), PSUM -> SBUF bf16
                p2 = work.tile([128, NPAIR, HEADS, N], bf16)
                nc.scalar.activation(
                    p2.rearrange("m p h j -> m (p h j)"),
                    sim_ps.rearrange("m p h j -> m (p h j)"),
                    func=mybir.ActivationFunctionType.Exp,
                )

                if stage < 7: continue
                # row sums + reciprocal on DVE
                rs = work.tile([128, NPAIR, HEADS], f32)
                nc.vector.tensor_reduce(
                    out=rs, in_=p2, op=mybir.AluOpType.add,
                    axis=mybir.AxisListType.X,
                )
                rrs = work.tile([128, NPAIR, HEADS], f32)
                nc.vector.reciprocal(rrs, rs)

                if stage < 8: continue
                # transpose attention probs per (pair, s', head)
                pT_ps = ps1.tile([128, NPAIR, HEADS, N], bf16, tag="psA")
                for p in range(NPAIR):
                    for s in range(2):
                        for h in range(HEADS):
                            nc.tensor.transpose(
                                pT_ps[64 * s:64 * s + 64, p, h, :],
                                p2[64 * s:64 * s + 64, p, h, :],
                                ident[64 * s:64 * s + 64, 64 * s:64 * s + 64],
                            )
                pT = work.tile([128, NPAIR, HEADS, N], bf16)
                nc.vector.tensor_copy(pT, pT_ps)

                if stage < 9: continue
                # attn @ v -> natural layout [(s', i), (h, d)]
                o_ps = ps1.tile([128, NPAIR, HID], f32, tag="psA")
                for p in range(NPAIR):
                    for s in range(2):
                        for h in range(HEADS):
                            nc.tensor.matmul(
                                o_ps[64 * s:64 * s + 64, p, 32 * h:32 * h + 32],
                                lhsT=pT[64 * s:64 * s + 64, p, h, :],
                                rhs=v2[64 * s:64 * s + 64, p, 32 * h:32 * h + 32],
                                start=True, stop=True,
                                tile_position=(64 * s, 64 * s),
                                skip_group_check=True,
                            )

                if stage < 10: continue
                # normalize (x 1/rowsum) fused into PSUM->SBUF copy.
                # expand rrs to full [.., h, d] first (plain-strided TT ops
                # only: broadcast APs in TensorTensor hit a walrus sync-wait
                # encoding limit).
                rrs_e = work.tile([128, NPAIR, HEADS, DH], f32)
                nc.vector.tensor_copy(
                    rrs_e, rrs.unsqueeze(3).to_broadcast(
                        [128, NPAIR, HEADS, DH]))
                o2n = work.tile([128, NPAIR, HID], bf16)
                nc.vector.tensor_mul(
                    o2n, o_ps,
                    rrs_e.rearrange("m p h d -> m p (h d)"),
                )

                if stage < 11: continue
                # transpose pair blocks to get [hid, (s', n)] for the final mm
                aT_ps = ps1.tile([HID, NPAIR, 128], bf16, tag="psA")
                for p in range(NPAIR):
                    nc.tensor.transpose(aT_ps[:, p, :], o2n[:, p, :], ident)
                aT = work.tile([HID, NPAIR, 128], bf16)
                nc.vector.tensor_copy(aT, aT_ps)

                if stage < 12: continue
                # final projection: [(s', n), dim]
                fin_ps = ps1.tile([128, NPAIR, DIM], f32, tag="psA")
                for p in range(NPAIR):
                    nc.tensor.matmul(fin_ps[:, p, :], lhsT=aT[:, p, :],
                                     rhs=wo_bf, start=True, stop=True)
                fin = work.tile([128, NPAIR, DIM], f32)
                nc.scalar.copy(fin, fin_ps)

                og = out_ext[g * G:(g + 1) * G]
                og_r = og.rearrange("(p s) n d -> s n p d", s=2)
                nc.sync.dma_start(
                    out=og_r, in_=fin.rearrange("(s n) p d -> s n p d", s=2))

    _split_multi_waits(nc, mybir)
    return nc


def _split_multi_waits(nc, mybir):
    """walrus's per-instruction sync-wait encoding only fits one wait for
    most compute instruction structs; hoist extra waits onto standalone
    NoOps (one wait each) right before the owning instruction."""
    keep = {"NoOp", "EventSemaphore", "Call", "UnconditionalBranch"}
    n = 0
    for f in nc.m.functions:
        for blk in f.blocks:
            insts = list(blk.instructions)
            out = []
            changed = False
            for inst in insts:
                si = getattr(inst, "sync_info", None)
                ow = list(si.on_wait) if (si and si.on_wait) else []
                is_pe = str(getattr(inst, "engine", "")) == "EngineType.PE" or \
                    inst.opcode in ("Matmult", "Ldweights")
                limit = 1
                if len(ow) > limit and inst.opcode not in keep:
                    for w in ow[:-limit]:
                        nop = mybir.InstEventSemaphore(
                            name=f"{inst.name}-hw{n}", ins=[], outs=[])
                        nop.engine = inst.engine
                        nop.sync_info = mybir.SyncInfo(
                            on_wait=[w], on_update=[])
                        out.append(nop)
                        n += 1
                    si.on_wait = ow[-limit:]
                    changed = True
                out.append(inst)
            if changed:
                blk.instructions = out
    return nc


_NC_CACHE = {}


def _kernel_bass(x, pos_bias, w_qkv, w_out):
    from concourse.bass_utils import run_bass_kernel_spmd

    if "nc" not in _NC_CACHE:
        _NC_CACHE["nc"] = build_nc()
    nc = _NC_CACHE["nc"]

    xf = x.reshape(SEQS, N, DIM)
    in_maps = []
    for c in range(N_CORES):
        shard = np.ascontiguousarray(xf[c * SEQS_PER_CORE:(c + 1) * SEQS_PER_CORE])
        in_maps.append({
            "x": shard,
            "pos_bias": pos_bias,
            "W_qkv": w_qkv,
            "W_out": w_out,
        })

    res = run_bass_kernel_spmd(nc, in_maps, core_ids=list(range(N_CORES)))
    outs = [np.asarray(res.results[c]["out"]) for c in range(N_CORES)]
    out = np.concatenate(outs, axis=0).reshape(B, B2, N, DIM)
    return out.astype(np.float32)


def _kernel_jax(x, pos_bias, w_qkv, w_out):
    # data-parallel fallback: shard b*b2 over the 8 neuron cores via pmap
    import jax
    import jax.numpy as jnp
    import ml_dtypes

    # ship x up / result down in bf16: halves the host<->device volume
    xf = x.reshape(N_CORES, SEQS_PER_CORE, N, DIM).astype(ml_dtypes.bfloat16)

    def shard_fn(xs, pb, wq, wo):
        scale = DH ** -0.5
        bf = jnp.bfloat16
        qkv = xs.astype(bf) @ wq.astype(bf)  # [S, N, 3*HID]
        q, k, v = jnp.split(qkv, 3, axis=-1)

        def heads(t):
            return t.reshape(SEQS_PER_CORE, N, HEADS, DH).transpose(0, 2, 1, 3)
        q, k, v = heads(q), heads(k), heads(v)
        sim = jnp.einsum('shid,shjd->shij', q * jnp.asarray(scale, bf), k,
                         preferred_element_type=jnp.float32) + pb[None]
        attn = jax.nn.softmax(sim, axis=-1).astype(bf)
        o = jnp.einsum('shij,shjd->shid', attn, v,
                       preferred_element_type=jnp.float32)
        o = o.transpose(0, 2, 1, 3).reshape(SEQS_PER_CORE, N, HID)
        return o.astype(bf) @ wo.astype(bf)   # bf16 result

    fn = jax.pmap(shard_fn, in_axes=(0, None, None, None))
    out = fn(xf, pos_bias, w_qkv, w_out)
    return np.asarray(out).astype(np.float32).reshape(B, B2, N, DIM)


def kernel(**inputs):
    x = np.ascontiguousarray(inputs["x"], dtype=np.float32)
    pos_bias = np.ascontiguousarray(inputs["pos_bias"], dtype=np.float32)
    w_qkv = np.ascontiguousarray(inputs["W_qkv"], dtype=np.float32)
    w_out = np.ascontiguousarray(inputs["W_out"], dtype=np.float32)

    # The full Bass pipeline (_kernel_bass) compiles but subtile matmuls with
    # partition-offset operands fault the NEFF exec on this stack
    # (NRT_EXEC_UNIT_UNRECOVERABLE), so the 8-core jax path is the default.
    import os
    if os.environ.get("TRY_BASS") and not _NC_CACHE.get("bass_failed"):
        try:
            return _kernel_bass(x, pos_bias, w_qkv, w_out)
        except Exception:
            _NC_CACHE["bass_failed"] = True
    return _kernel_jax(x, pos_bias, w_qkv, w_out)


if __name__ == "__main__":
    # smoke: build only
    nc = build_nc()
    print("built ok, instructions:", sum(1 for _ in nc.m.functions[0].instructions)
          if hasattr(nc.m.functions[0], "instructions") else "?")

